# revision 11
# baseline (speedup 1.0000x reference)
"""AttentiveFPConv GNN message-passing kernel for 8 Trainium2 NeuronCores.

Reference computation (all fp32):
    alpha = sigmoid(x[col] @ Wa_w + Wa_b)          # per-edge attention
    neigh = x[col] * alpha                          # per-edge message
    aggr  = segment_sum(neigh, row, N)              # per-node aggregation
    out   = tanh(x @ Wn_w + Wn_b + aggr @ Wg_w + Wg_b)

Key algebraic identity: alpha depends only on the source node, so
    h = x * sigmoid(x @ Wa_w + Wa_b)                # per-NODE tensor
    aggr[n] = sum_{e: row[e]=n} h[col[e]]           # gather + segment-sum

Sharding: destination-node sharding. Core k owns nodes [5000k, 5000(k+1))
and ALL edges targeting them (balanced: rows are uniform). No collective
needed: each core computes its own aggr and output slice.

Per-core pipeline:
  Phase 1: h = x*sigmoid(x@Wa+b) for ALL nodes (replicated), h -> HBM bf16.
  Phase 2: dma_gather h[col] in destination-sorted edge order (4 SWDGE
           queues); segment-sum via one-hot matmuls accumulating aggr^T in
           PSUM per 128-node block. One-hot M built by DVE tensor_scalar
           reading a PSUM-resident iota (1-port mode: avoids the exclusive
           DVE<->GpSimd shared-SBUF-port lock that otherwise serializes
           against Q7 gather descriptor generation).
           (dma_gather indices are int16, so edges are split into two
           streams by col < 32768, each gathered against a rebased view.)
  Phase 3: out = tanh(x@Wn + aggr@Wg + ones x bias) -- bias added by a
           rank-1 matmul into the same PSUM accumulation group.
"""

import numpy as np
import ml_dtypes

BF16 = ml_dtypes.bfloat16

# ---------------------------------------------------------------- parameters

class P:
    """Problem/kernel parameters (full-size defaults; shrinkable for tests)."""
    def __init__(self, N=40000, D=128, NCORES=8, HSPLIT=19968,
                 GCHUNK=1024, PH1_CHUNK=2048, NQ=4):
        assert D == 128
        self.N, self.D, self.NCORES = N, D, NCORES
        self.NB = N // NCORES                 # nodes per core
        self.HSPLIT = HSPLIT                  # col split for int16 gather idx
        self.GCHUNK = GCHUNK                  # idxs per dma_gather (HW limit ~1024)
        self.GT = GCHUNK // 128               # gather tiles per chunk
        self.PH1_CHUNK = PH1_CHUNK            # nodes per phase-1 xT chunk
        self.NBLK = (self.NB + 127) // 128    # 128-node blocks per core
        self.NQ = NQ                          # SWDGE queues for dma_gather


# ------------------------------------------------------------ host edge prep

def prep_edges(p: P, row: np.ndarray, col: np.ndarray):
    """Per-core destination-sorted, block-padded edge streams."""
    row = np.asarray(row).astype(np.int64)
    col = np.asarray(col).astype(np.int64)
    cores = []
    for k in range(p.NCORES):
        sel = (row // p.NB) == k
        r = (row[sel] - k * p.NB).astype(np.int32)
        c = col[sel].astype(np.int32)
        order = np.argsort(r, kind="stable")
        r, c = r[order], c[order]
        lo = np.searchsorted(r, np.arange(p.NBLK) * 128)
        hi = np.searchsorted(r, np.minimum(np.arange(1, p.NBLK + 1) * 128, p.NB))
        blocks = []
        for b in range(p.NBLK):
            rb = r[lo[b]:hi[b]] - b * 128
            cb = c[lo[b]:hi[b]]
            mA = cb < p.HSPLIT
            blocks.append(((cb[mA], rb[mA]), (cb[~mA] - p.HSPLIT, rb[~mA])))
        cores.append(blocks)

    nA = np.array([[len(cores[k][b][0][0]) for b in range(p.NBLK)]
                   for k in range(p.NCORES)])
    nB = np.array([[len(cores[k][b][1][0]) for b in range(p.NBLK)]
                   for k in range(p.NCORES)])
    tA = np.maximum(1, -(-nA.max(axis=0) // 128))          # [NBLK]
    tB = np.maximum(1, -(-nB.max(axis=0) // 128))

    LA, LB = int(tA.sum()) * 128, int(tB.sum()) * 128
    LAg = -(-LA // p.GCHUNK) * p.GCHUNK
    LBg = -(-LB // p.GCHUNK) * p.GCHUNK

    per_core = []
    for k in range(p.NCORES):
        idxA = np.zeros(LAg, np.int16); lrA = np.full(LA, -1.0, np.float32)
        idxB = np.zeros(LBg, np.int16); lrB = np.full(LB, -1.0, np.float32)
        oA = oB = 0
        for b in range(p.NBLK):
            (cA, rA), (cB, rB) = cores[k][b]
            idxA[oA:oA + len(cA)] = cA; lrA[oA:oA + len(rA)] = rA
            oA += int(tA[b]) * 128
            idxB[oB:oB + len(cB)] = cB; lrB[oB:oB + len(rB)] = rB
            oB += int(tB[b]) * 128
        per_core.append({
            "idxA": np.tile(idxA.reshape(-1, 16).T, (8, 1)),   # [128, LAg/16]
            "idxB": np.tile(idxB.reshape(-1, 16).T, (8, 1)),
            "lrA": lrA.reshape(-1, 128).T.copy(),              # [128, LA/128]
            "lrB": lrB.reshape(-1, 128).T.copy(),
        })
    return tA, tB, LA, LB, LAg, LBg, per_core


# ------------------------------------------------------------- device kernel

def build(p: P, tA, tB, LA, LB, LAg, LBg):
    from concourse import bacc, mybir, tile

    f32, bf16, i16 = mybir.dt.float32, mybir.dt.bfloat16, mybir.dt.int16
    AF = mybir.ActivationFunctionType
    nc = bacc.Bacc("TRN2", target_bir_lowering=False, debug=False,
                   num_devices=p.NCORES, num_swdge_queues=p.NQ)

    N, D, NB, NBLK = p.N, p.D, p.NB, p.NBLK
    H = p.HSPLIT                    # h1 rows; h2 rows = N - H
    N2 = N - H
    assert H % 128 == 0

    xT_d   = nc.dram_tensor("xT", [D, N], bf16, kind="ExternalInput")
    xTo_d  = nc.dram_tensor("xT_own", [D, NB], bf16, kind="ExternalInput")
    WaW_d  = nc.dram_tensor("WaW", [D, D], bf16, kind="ExternalInput")
    WaB_d  = nc.dram_tensor("WaB", [D, 1], f32, kind="ExternalInput")
    WnW_d  = nc.dram_tensor("WnW", [D, D], bf16, kind="ExternalInput")
    WgW_d  = nc.dram_tensor("WgW", [D, D], bf16, kind="ExternalInput")
    bias_d = nc.dram_tensor("biasR", [1, D], bf16, kind="ExternalInput")
    ones_d = nc.dram_tensor("onesR", [1, D], bf16, kind="ExternalInput")
    ident_d= nc.dram_tensor("ident", [D, D], bf16, kind="ExternalInput")
    idxA_d = nc.dram_tensor("idxA", [128, LAg // 16], i16, kind="ExternalInput")
    idxB_d = nc.dram_tensor("idxB", [128, LBg // 16], i16, kind="ExternalInput")
    MA_d   = nc.dram_tensor("MA", [128, LA // 128, D], bf16, kind="ExternalInput")
    MB_d   = nc.dram_tensor("MB", [128, LB // 128, D], bf16, kind="ExternalInput")
    out_d  = nc.dram_tensor("out", [NB, D], f32, kind="ExternalOutput")
    h1_d   = nc.dram_tensor("h1", [H, D], bf16, kind="Internal")
    h2_d   = nc.dram_tensor("h2", [N2, D], bf16, kind="Internal")

    PIECE = 9984                   # nodes per hT staging piece (78 blocks)

    with tile.TileContext(nc) as tc:
        with (
            tc.tile_pool(name="const", bufs=1) as cpool,
            tc.tile_pool(name="xchunk", bufs=4) as xpool,
            tc.tile_pool(name="hT", bufs=2) as htpool,
            tc.tile_pool(name="hstage", bufs=1) as hspool,
            tc.tile_pool(name="pg", bufs=2, space="PSUM") as pg_pool,
            tc.tile_pool(name="pt", bufs=1, space="PSUM") as pt_pool,
            tc.tile_pool(name="pa", bufs=2, space="PSUM") as pa_pool,
            tc.tile_pool(name="po", bufs=2, space="PSUM") as po_pool,
            tc.tile_pool(name="sA", bufs=12) as gApool,
            tc.tile_pool(name="sB", bufs=12) as gBpool,
            tc.tile_pool(name="m", bufs=6) as mpool,
            tc.tile_pool(name="aggA", bufs=(NBLK + 3) // 4) as aggApool,
            tc.tile_pool(name="aggB", bufs=3) as aggBpool,
            tc.tile_pool(name="ph1w", bufs=4) as w1pool,
            tc.tile_pool(name="ostage", bufs=2) as ospool,
        ):
            # ---- constants into SBUF
            WaW = cpool.tile([D, D], bf16); nc.sync.dma_start(out=WaW[:], in_=WaW_d[:])
            WaB = cpool.tile([D, 1], f32); nc.sync.dma_start(out=WaB[:], in_=WaB_d[:])
            WnW = cpool.tile([D, D], bf16); nc.sync.dma_start(out=WnW[:], in_=WnW_d[:])
            WgW = cpool.tile([D, D], bf16); nc.sync.dma_start(out=WgW[:], in_=WgW_d[:])
            biasR = cpool.tile([1, D], bf16); nc.sync.dma_start(out=biasR[:], in_=bias_d[:])
            onesR = cpool.tile([1, D], bf16); nc.sync.dma_start(out=onesR[:], in_=ones_d[:])
            ident = cpool.tile([D, D], bf16); nc.sync.dma_start(out=ident[:], in_=ident_d[:])
            xT_own = cpool.tile([D, NB], bf16); nc.sync.dma_start(out=xT_own[:], in_=xTo_d[:])
            idxA_sb = cpool.tile([128, LAg // 16], i16)
            nc.sync.dma_start(out=idxA_sb[:], in_=idxA_d[:])
            idxB_sb = cpool.tile([128, LBg // 16], i16)
            nc.sync.dma_start(out=idxB_sb[:], in_=idxB_d[:])


            # ---- phase 1: h = x * sigmoid(x@Wa + b); hT pieces -> xbar -> HBM
            def ph1_compute(hTp, base, cn):
                """Compute hT for nodes [base, base+cn) into hTp[:, :cn]."""
                off = 0
                while off < cn:
                    w = min(2048, cn - off)
                    xc = xpool.tile([D, 2048], bf16, tag="xc")
                    nc.sync.dma_start(out=xc[:, :w], in_=xT_d[:, base + off:base + off + w])
                    g0 = 0
                    while g0 < w:
                        gw = min(512, w - g0)
                        pg = pg_pool.tile([D, 512], f32, tag="pg")
                        nc.tensor.matmul(pg[:, :gw], lhsT=WaW[:],
                                         rhs=xc[:, g0:g0 + gw], start=True, stop=True)
                        sT = w1pool.tile([D, 512], bf16, tag="sT")
                        nc.scalar.activation(sT[:, :gw], pg[:, :gw], AF.Sigmoid,
                                             bias=WaB[:, 0:1])
                        nc.vector.tensor_tensor(out=hTp[:, off + g0:off + g0 + gw],
                                                in0=xc[:, g0:g0 + gw],
                                                in1=sT[:, :gw], op=mybir.AluOpType.mult)
                        g0 += gw
                    off += w

            def ph1_flush(hTp, h_t, base_in_h, cn):
                """xbar-transpose hTp[:, :cn] and DMA to h_t rows [base_in_h, +cn)."""
                nfull = cn // 128
                rem = cn - nfull * 128
                if nfull:
                    hst = hspool.tile([128, PIECE // 128, 128], bf16, tag="hst")
                    nc.sync.dma_start_transpose(hst[:, :nfull, :], hTp[:, :nfull * 128])
                    nc.sync.dma_start(
                        out=h_t[base_in_h:base_in_h + nfull * 128, :].rearrange(
                            "(t p) d -> p t d", p=128),
                        in_=hst[:, :nfull, :])
                if rem:
                    pt = pt_pool.tile([128, 128], bf16, tag="pt")
                    nc.tensor.transpose(pt[:rem, :], hTp[:, nfull * 128:nfull * 128 + rem],
                                        ident[:])
                    tl = w1pool.tile([128, 128], bf16, tag="tail")
                    nc.vector.tensor_copy(out=tl[:rem, :], in_=pt[:rem, :])
                    nc.sync.dma_start(
                        out=h_t[base_in_h + nfull * 128:base_in_h + cn, :],
                        in_=tl[:rem, :])

            # h1: nodes [0, H)
            base = 0
            while base < H:
                cn = min(PIECE, H - base)
                hTp = htpool.tile([D, PIECE], bf16, tag="hT")
                ph1_compute(hTp, base, cn)
                ph1_flush(hTp, h1_d, base, cn)
                base += cn
            # h2: nodes [H, N)
            while base < N:
                cn = min(PIECE, N - base)
                hTp = htpool.tile([D, PIECE], bf16, tag="hT")
                ph1_compute(hTp, base, cn)
                ph1_flush(hTp, h2_d, base - H, cn)
                base += cn

            # ---- phase 2: two passes (A from h1, B from h2), one-hot scatter
            nq_counter = [0]
            gA_tiles = [None] * (LAg // p.GCHUNK)
            gB_tiles = [None] * (LBg // p.GCHUNK)
            mA_tiles = [None] * (-(-(LA // 128) // p.GT))
            mB_tiles = [None] * (-(-(LB // 128) // p.GT))

            def ensure_chunk(tiles, which, ci):
                if tiles[ci] is not None:
                    return
                g = (gApool if which == "A" else gBpool).tile(
                    [128, p.GT, D], bf16, tag="g" + which)
                idx_sb = idxA_sb if which == "A" else idxB_sb
                src = h1_d[:, :] if which == "A" else h2_d[:, :]
                c0 = ci * (p.GCHUNK // 16)
                nc.gpsimd.dma_gather(
                    out_ap=g[:], in_ap=src, idxs_ap=idx_sb[:, c0:c0 + p.GCHUNK // 16],
                    num_idxs=p.GCHUNK, num_idxs_reg=p.GCHUNK, elem_size=D,
                    queue_num=nq_counter[0] % p.NQ)
                nq_counter[0] += 1
                tiles[ci] = g

            def ensure_mchunk(tiles, which, ci):
                if tiles[ci] is not None:
                    return
                md = MA_d if which == "A" else MB_d
                nt = md.shape[1]
                t0 = ci * p.GT
                tn = min(p.GT, nt - t0)
                mt = mpool.tile([128, p.GT, D], bf16, tag="m" + which)
                nc.sync.dma_start(out=mt[:, :tn, :], in_=md[:, t0:t0 + tn, :])
                tiles[ci] = mt

            posA = np.concatenate([[0], np.cumsum(tA)]).astype(int)
            posB = np.concatenate([[0], np.cumsum(tB)]).astype(int)
            NG = (NBLK + 3) // 4

            def scatter_pass(tcnts, pos, gtiles, mtiles, which, aggpool):
                """One-hot matmul scatter for one stream; returns agg tiles."""
                aggs = []
                b0 = 0
                while b0 < NBLK:
                    gn = min(4, NBLK - b0)
                    pa = pa_pool.tile([D, 512], f32, tag="pa")
                    for q in range(gn):
                        b = b0 + q
                        tcnt = int(tcnts[b])
                        for j in range(tcnt):
                            g = pos[b] + j
                            ensure_chunk(gtiles, which, g // p.GT)
                            ensure_mchunk(mtiles, which, g // p.GT)
                            neigh = gtiles[g // p.GT][:, g % p.GT, :]
                            M = mtiles[g // p.GT][:, g % p.GT, :]
                            nc.tensor.matmul(pa[:, q * 128:(q + 1) * 128],
                                             lhsT=neigh, rhs=M,
                                             start=(j == 0), stop=(j == tcnt - 1))
                    agg = aggpool.tile([D, 512], bf16, tag="agg" + which)
                    nc.vector.tensor_copy(out=agg[:], in_=pa[:])
                    aggs.append(agg)
                    b0 += gn
                return aggs

            aggsA = scatter_pass(tA, posA, gA_tiles, mA_tiles, "A", aggApool)

            # ---- pass B + phase 3 fused per 4-block group
            OCH = 8
            ost = None
            ost_base = 0
            ost_n = 0
            b0 = 0
            while b0 < NBLK:
                gn = min(4, NBLK - b0)
                gi = b0 // 4
                pa = pa_pool.tile([D, 512], f32, tag="pa")
                for q in range(gn):
                    b = b0 + q
                    tcnt = int(tB[b])
                    for j in range(tcnt):
                        g = posB[b] + j
                        ensure_chunk(gB_tiles, "B", g // p.GT)
                        ensure_mchunk(mB_tiles, "B", g // p.GT)
                        neigh = gB_tiles[g // p.GT][:, g % p.GT, :]
                        M = mB_tiles[g // p.GT][:, g % p.GT, :]
                        nc.tensor.matmul(pa[:, q * 128:(q + 1) * 128],
                                         lhsT=neigh, rhs=M,
                                         start=(j == 0), stop=(j == tcnt - 1))
                aggB = aggBpool.tile([D, 512], bf16, tag="aggB")
                nc.vector.tensor_copy(out=aggB[:], in_=pa[:])

                po = po_pool.tile([128, 512], f32, tag="po")
                for q in range(gn):
                    b = b0 + q
                    nb = min(128, NB - b * 128)
                    sl = slice(q * 128, q * 128 + D)
                    nc.tensor.matmul(po[:nb, sl],
                                     lhsT=xT_own[:, b * 128:b * 128 + nb],
                                     rhs=WnW[:], start=True, stop=False)
                    nc.tensor.matmul(po[:nb, sl],
                                     lhsT=aggsA[gi][:, q * 128:q * 128 + nb],
                                     rhs=WgW[:], start=False, stop=False)
                    nc.tensor.matmul(po[:nb, sl],
                                     lhsT=aggB[:, q * 128:q * 128 + nb],
                                     rhs=WgW[:], start=False, stop=False)
                    nc.tensor.matmul(po[:nb, sl], lhsT=onesR[:1, :nb],
                                     rhs=biasR[:1, :], start=False, stop=True)
                for q in range(gn):
                    b = b0 + q
                    nb = min(128, NB - b * 128)
                    if ost is None:
                        ost = ospool.tile([128, OCH * D], f32, tag="ost")
                        ost_base = b
                        ost_n = 0
                    nc.scalar.activation(ost[:nb, ost_n * D:(ost_n + 1) * D],
                                         po[:nb, q * 128:q * 128 + D], AF.Tanh)
                    ost_n += 1
                    if (ost_n == OCH) or (b == NBLK - 1):
                        rows0 = ost_base * 128
                        nfull_o = ost_n if nb == 128 else ost_n - 1
                        if nfull_o:
                            nc.sync.dma_start(
                                out=out_d[rows0:rows0 + nfull_o * 128, :].rearrange(
                                    "(t p) d -> p t d", p=128),
                                in_=ost[:, :nfull_o * D].rearrange(
                                    "p (t d) -> p t d", d=D))
                        if nb != 128:
                            nc.sync.dma_start(
                                out=out_d[rows0 + nfull_o * 128:
                                          rows0 + nfull_o * 128 + nb, :],
                                in_=ost[:nb, nfull_o * D:nfull_o * D + D])
                        ost = None
                b0 += gn

    nc.compile()
    return nc


# ---------------------------------------------------------------- host entry

def _host_prep(p: P, x, edge_index, Wn_w, Wn_b, Wg_w, Wg_b, Wa_w, Wa_b):
    x = np.asarray(x, np.float32)
    xT = np.ascontiguousarray(x.T).astype(BF16)
    tA, tB, LA, LB, LAg, LBg, per_core = prep_edges(
        p, np.asarray(edge_index)[0], np.asarray(edge_index)[1])

    shared = {
        "xT": xT,
        "WaW": np.asarray(Wa_w, np.float32).astype(BF16),
        "WaB": np.asarray(Wa_b, np.float32).reshape(p.D, 1),
        "WnW": np.asarray(Wn_w, np.float32).astype(BF16),
        "WgW": np.asarray(Wg_w, np.float32).astype(BF16),
        "biasR": (np.asarray(Wn_b, np.float32)
                  + np.asarray(Wg_b, np.float32)).reshape(1, p.D).astype(BF16),
        "onesR": np.ones((1, p.D), BF16),
        "ident": np.eye(p.D, dtype=np.float32).astype(BF16),
    }
    in_maps = []
    for k in range(p.NCORES):
        m = dict(shared)
        m["xT_own"] = np.ascontiguousarray(xT[:, k * p.NB:(k + 1) * p.NB])
        pc = per_core[k]
        m["idxA"], m["idxB"] = pc["idxA"], pc["idxB"]
        jj = np.arange(p.D, dtype=np.float32)[None, None, :]
        m["MA"] = (pc["lrA"][:, :, None] == jj).astype(BF16)
        m["MB"] = (pc["lrB"][:, :, None] == jj).astype(BF16)
        in_maps.append(m)
    return tA, tB, LA, LB, LAg, LBg, in_maps


TRACE = False      # set True (e.g. from test.py) to capture an NTFF profile
LAST = None        # last BassKernelResults, for profiling/inspection


def kernel(**inputs) -> np.ndarray:
    global LAST
    from concourse import bass_utils
    bass_utils.upload_artifacts = lambda tmpdir: "local://" + tmpdir

    p = P()
    tA, tB, LA, LB, LAg, LBg, in_maps = _host_prep(p, **inputs)
    nc = build(p, tA, tB, LA, LB, LAg, LBg)
    kw = dict(trace=True, trace_cores=list(range(p.NCORES))) if TRACE else {}
    res = bass_utils.run_bass_kernel_spmd(
        nc, in_maps, core_ids=list(range(p.NCORES)), **kw)
    LAST = res
    out = np.concatenate([res.results[k]["out"] for k in range(p.NCORES)], axis=0)
    return out.astype(np.float32)
